# revision 1
# baseline (speedup 1.0000x reference)
"""AdvancedStateBank Trainium2 kernel (8-core SPMD, token-parallel).

Each core handles 256 of the 2048 tokens; K/V banks and MLP weights are
replicated, so no collectives are needed. Key choices:

- Scores q@K^T run as bf16x3 (hi/lo split of both operands, dropping only
  the lo*lo term): ~1e-7 absolute error, preserving exact top-k membership
  at 3 PE-cycles/row instead of f32's 4. K is transposed on the fly through
  the PE (batched 4-per-PSUM-bank with a single wide evacuation).
- Top-64 per token: per-512-column-chunk top-32 via vector.max +
  match_replace rounds (P[a chunk holds >32 of a row's top-64] ~ 1e-7),
  merged by a final 8-round pass. Chunk rounds are emitted inline with the
  scores chunks so the DVE starts ~30us into the kernel.
- Dynamic-k needs no floor(): valid[j] = (j+1 <= 64*sigmoid(.)), and the
  selection threshold is the smallest valid entry of the sorted top-64, so
  the selected set matches jax.lax.top_k + clipped-floor exactly.
- Masked softmax in place: exp(s-max) overwrites the match_replace scratch,
  one scalar_tensor_tensor masks it and accumulates the denominator; route
  weight and 1/denom fold into the output accumulation.
- probs@V runs in bf16 (probsT produced by PE transpose with a rounding
  PSUM evacuation; V converted on Pool/ACT while idle), N=512 moving.
- The three levels are software-pipelined (scores of later levels emitted
  before the top-k/read phase of earlier ones) so the PE stream never
  blocks behind the DVE top-k chain.
"""
import sys

if "/opt/trn_rl_repo" not in sys.path:
    sys.path.insert(0, "/opt/trn_rl_repo")

import numpy as np

import concourse.bacc as bacc
import concourse.mybir as mybir
from concourse.tile import TileContext
from concourse.bass_utils import run_bass_kernel_spmd
from concourse.masks import make_identity

F32 = mybir.dt.float32
F32R = mybir.dt.float32r
BF16 = mybir.dt.bfloat16
U8 = mybir.dt.uint8
AF = mybir.ActivationFunctionType
ALU = mybir.AluOpType
AX = mybir.AxisListType

D = 1024
B, T = 2, 1024
SLOTS = [2048, 1024, 512]
MAXK = 64
N_CORES = 8
TOK_PER_CORE = (B * T) // N_CORES  # 256
TT = TOK_PER_CORE // 128           # 2 token tiles per core
SCALE = float(1.0 / np.sqrt(np.float32(D)))
NEG_BIG = -1.0e30


def _rounds(nc, dst, src, scr, nr=8):
    # dst[:, r*8:(r+1)*8] <- r-th top-8 batch (descending); scr is clobbered
    nc.vector.max(out=dst[:, 0:8], in_=src)
    if nr == 1:
        return
    nc.vector.match_replace(out=scr, in_to_replace=dst[:, 0:8],
                            in_values=src, imm_value=NEG_BIG)
    for r in range(1, nr):
        nc.vector.max(out=dst[:, r * 8:(r + 1) * 8], in_=scr)
        if r < nr - 1:
            nc.vector.match_replace(
                out=scr, in_to_replace=dst[:, r * 8:(r + 1) * 8],
                in_values=scr, imm_value=NEG_BIG)


def _emit_body(nc, env, rep):
    """One full inference over this core's 256 tokens."""
    cp, pp, wp, strm, sm = env["cp"], env["pp"], env["wp"], env["strm"], env["sm"]
    ps_tp, ps_s, ps_rd, ps_sm = (env["ps_tp"], env["ps_s"], env["ps_rd"],
                                 env["ps_sm"])
    ident, ones, iota_f, big = env["ident"], env["ones"], env["iota_f"], env["big"]
    rb1_row, rb2_row, pb1_row, pb2_row = (env["rb1_row"], env["rb2_row"],
                                          env["pb1_row"], env["pb2_row"])
    pW1_sb, pW2_sb, rW2_sb = env["pW1_sb"], env["pW2_sb"], env["rW2_sb"]
    qT, out_sb, route_sb, valid = (env["qT"], env["out_sb"], env["route_sb"],
                                   env["valid"])
    q, Ks, Vs, sals, rW1, out = (env["q"], env["Ks"], env["Vs"], env["sals"],
                                 env["rW1"], env["out"])

    qTbs = []
    qTlos = []

    # ---------------- q load/transpose ----------------
    for t in range(TT):
        qhs = []
        for g in range(2):
            qh = wp.tile([128, 512], F32, tag=f"qsb{g}", bufs=1,
                         name=f"q_{rep}_{t}_{g}")
            nc.sync.dma_start(qh, q[t * 128:(t + 1) * 128,
                                    g * 512:(g + 1) * 512])
            qhs.append(qh)
        for g in range(2):
            pst = ps_tp.tile([128, 512], F32, tag="tp", name="pst")
            for j in range(4):
                nc.tensor.transpose(pst[:, j * 128:(j + 1) * 128],
                                    qhs[g][:, j * 128:(j + 1) * 128], ident)
            nc.scalar.copy(qT[t][:, g * 4:(g + 1) * 4, :], pst)
        qTb = wp.tile([128, 8, 128], BF16, tag=f"qTb{t}", name=f"qTb_{rep}_{t}")
        nc.vector.tensor_copy(qTb, qT[t])
        qTbs.append(qTb)
        qTl = wp.tile([128, 8, 128], BF16, tag=f"qTl{t}", name=f"qTl_{rep}_{t}")
        nc.vector.tensor_tensor(out=qTl, in0=qT[t], in1=qTb, op=ALU.subtract)
        qTlos.append(qTl)

    # ---------------- router: h1 = gelu(q @ rW1 + rb1) ----------------
    # h1 in [T, Dout] orientation (rW1 used as stored, streamed), then
    # PE-transposed to h1T for the second matmul.
    def router_phase():
      ph1 = {}
      for t in range(TT):
          for c2 in range(2):
              ph1[(t, c2)] = ps_rd.tile([128, 512], F32, tag=f"rd{t}{c2}",
                                        name=f"ph1_{rep}_{t}{c2}", bufs=1)
      for ci in range(8):
          rwc = strm.tile([128, D], F32, tag="stream", bufs=4,
                          name=f"rwc_{rep}_{ci}")
          nc.sync.dma_start(rwc, rW1[ci * 128:(ci + 1) * 128, :])
          rwb = strm.tile([128, D], BF16, tag="rwb", bufs=2,
                          name=f"rwb_{rep}_{ci}")
          nc.gpsimd.tensor_copy(rwb, rwc)
          for t in range(TT):
              for c2 in range(2):
                  nc.tensor.matmul(ph1[(t, c2)], qTbs[t][:, ci, :],
                                   rwb[:, c2 * 512:(c2 + 1) * 512],
                                   start=(ci == 0), stop=False)
      for t in range(TT):
          for c2 in range(2):
              nc.tensor.matmul(ph1[(t, c2)], ones[32:33, :],
                               rb1_row[0:1, c2 * 512:(c2 + 1) * 512],
                               start=False, stop=True)
      for t in range(TT):
          h1_sb = wp.tile([128, D], F32, tag=f"scores{t}", bufs=2, name=f"h1_{rep}_{t}")
          for c2 in range(2):
              nc.scalar.activation(h1_sb[:, c2 * 512:(c2 + 1) * 512],
                                   ph1[(t, c2)], AF.Gelu)
          h1T = wp.tile([128, 8, 128], F32, tag="h1T", bufs=1,
                        name=f"h1T_{rep}_{t}")
          for g in range(2):
              pst = ps_tp.tile([128, 512], F32, tag="tp", name="pst")
              for j in range(4):
                  dc = g * 4 + j
                  nc.tensor.transpose(pst[:, j * 128:(j + 1) * 128],
                                      h1_sb[:, dc * 128:(dc + 1) * 128], ident)
              nc.scalar.copy(h1T[:, g * 4:(g + 1) * 4, :], pst)
          # route logits [T,3] + softmax over 3
          pr = ps_s.tile([128, 3], F32, tag="ps_s", name="pr")
          for co in range(8):
              nc.tensor.matmul(pr, h1T[:, co, :], rW2_sb[:, co, :],
                               start=(co == 0), stop=False)
          nc.tensor.matmul(pr, ones[0:1, :], rb2_row, start=False, stop=True)
          m3 = sm.tile([128, 1], F32, tag="m3", name="m3")
          nc.vector.tensor_reduce(m3, pr, axis=AX.X, op=ALU.max)
          negm3 = sm.tile([128, 1], F32, tag="negm3", name="negm3")
          nc.vector.tensor_scalar_mul(negm3, m3, -1.0)
          e3 = sm.tile([128, 3], F32, tag="e3", name="e3")
          nc.scalar.activation(e3, pr, AF.Exp, bias=negm3[:, 0:1])
          s3 = sm.tile([128, 1], F32, tag="s3", name="s3")
          nc.vector.tensor_reduce(s3, e3, axis=AX.X, op=ALU.add)
          inv3 = sm.tile([128, 1], F32, tag="inv3", name="inv3")
          nc.vector.reciprocal(inv3, s3)
          nc.vector.tensor_scalar(route_sb[t], e3, inv3[:, 0:1], None,
                                  op0=ALU.mult)

          # predictor: p1T = gelu(pW1^T @ q^T + pb1 x ones)  [64, T]
          pps = ps_s.tile([64, 128], F32, tag="ps_s", name="pps")
          for ci in range(8):
              nc.tensor.matmul(pps, pW1_sb[:, ci, :], qT[t][:, ci, :],
                               start=(ci == 0), stop=False)
          nc.tensor.matmul(pps, pb1_row, ones[0:1, :], start=False, stop=True)
          p1T = sm.tile([64, 128], F32, tag="p1T", name="p1T")
          nc.scalar.activation(p1T, pps, AF.Gelu)
          pl = ps_s.tile([128, 1], F32, tag="ps_s", name="pl")
          nc.tensor.matmul(pl, p1T, pW2_sb, start=True, stop=False)
          nc.tensor.matmul(pl, ones[0:1, :], pb2_row, start=False, stop=True)
          sg = sm.tile([128, 1], F32, tag="sg", name="sg")
          nc.scalar.activation(sg, pl, AF.Sigmoid)
          kx64 = sm.tile([128, 1], F32, tag="kx64", name="kx64")
          nc.vector.tensor_scalar_mul(kx64, sg, float(MAXK))
          # valid[j] = (iota_f[j] <= kx64); col0 = -1e30 => always true
          nc.vector.tensor_scalar(valid[t], iota_f, kx64[:, 0:1], None,
                                  op0=ALU.is_le)

    # ---------------- levels (software-pipelined) ----------------
    def scores_phase(li):
        S = SLOTS[li]
        nch = S // 512

        sal_row = env["sal_rows"][li]

        stag = f"scores2_{{}}" if li == 2 else f"scores{{}}"
        sbufs = 1 if li == 2 else 2
        scores_t = [wp.tile([128, S], F32, tag=stag.format(t), bufs=sbufs,
                            name=f"sco_{rep}_{li}_{t}") for t in range(TT)]
        split = (S >= 2048)
        cands = scratches = None
        if split:
            cands = [sm.tile([128, 32 * nch], F32, tag=f"cand{t}",
                             name=f"cand{t}") for t in range(TT)]
            scratches = [wp.tile([128, S], F32, tag=f"scratch{t}", bufs=2,
                                 name=f"scr_{rep}_{li}_{t}")
                         for t in range(TT)]

        # scores, with K transposed on the fly in 512-slot blocks; for the
        # split level each 512-chunk's top-64 rounds are emitted right after
        # its evac so DVE starts while later chunks are still on the PE.
        for ch in range(nch):
            ktb = pp.tile([128, 8, 512], F32, tag="ktb", name="ktb", bufs=1)
            ktbh = pp.tile([128, 8, 512], BF16, tag="ktbh", name="ktbh",
                           bufs=2)
            ktbl = pp.tile([128, 8, 512], BF16, tag="ktbl", name="ktbl",
                           bufs=2)
            ksbs = []
            for s4 in range(4):
                ksb = strm.tile([128, D], F32, tag="stream", bufs=4,
                                name="ksb")
                nc.sync.dma_start(
                    ksb,
                    Ks[li][ch * 512 + s4 * 128:ch * 512 + (s4 + 1) * 128, :])
                ksbs.append(ksb)
            use_f32 = False  # fewer handoffs on the critical prefix
            for dc in range(8):
                pst = ps_tp.tile([128, 512], F32, tag="tp", name="pst")
                for s4 in range(4):
                    nc.tensor.transpose(pst[:, s4 * 128:(s4 + 1) * 128],
                                        ksbs[s4][:, dc * 128:(dc + 1) * 128],
                                        ident)
                nc.scalar.mul(ktb[:, dc, :], pst, SCALE)
                if not use_f32:
                    nc.vector.tensor_copy(ktbh[:, dc, :], ktb[:, dc, :])
                    nc.gpsimd.tensor_tensor(out=ktbl[:, dc, :],
                                            in0=ktb[:, dc, :],
                                            in1=ktbh[:, dc, :],
                                            op=ALU.subtract)
            for t in range(TT):
                pss = ps_s.tile([128, 512], F32, tag="ps_s", bufs=2,
                                name="pss")
                if use_f32:
                    for ci in range(8):
                        nc.tensor.matmul(pss, qT[t][:, ci, :], ktb[:, ci, :],
                                         start=(ci == 0), stop=False)
                else:
                    # hi terms first: the group starts without waiting for
                    # the Pool-produced ktbl residual
                    for ci in range(8):
                        nc.tensor.matmul(pss, qTbs[t][:, ci, :],
                                         ktbh[:, ci, :],
                                         start=(ci == 0), stop=False)
                        nc.tensor.matmul(pss, qTlos[t][:, ci, :],
                                         ktbh[:, ci, :],
                                         start=False, stop=False)
                    for ci in range(8):
                        nc.tensor.matmul(pss, qTbs[t][:, ci, :],
                                         ktbl[:, ci, :],
                                         start=False, stop=False)
                nc.tensor.matmul(pss, ones[li * 32:li * 32 + 1, :],
                                 sal_row[0:1, ch * 512:(ch + 1) * 512],
                                 start=False, stop=True)
                nc.scalar.copy(scores_t[t][:, ch * 512:(ch + 1) * 512], pss)
            if split:
                # top-32 per 512-chunk is a statistically-safe superset of
                # each chunk's contribution to the row top-64
                # (P[Binom(64,1/4) > 32] ~ 1e-7 per row).
                for t in range(TT):
                    seg = slice(ch * 512, (ch + 1) * 512)
                    _rounds(nc, cands[t][:, ch * 32:(ch + 1) * 32],
                            scores_t[t][:, seg], scratches[t][:, seg], nr=4)
        return scores_t, cands, scratches

    def load_v(li):
        S = SLOTS[li]
        tiles = []
        for sc in range(S // 128):
            vsb = strm.tile([128, D], F32, tag="stream", bufs=4, name="vsb")
            nc.sync.dma_start(vsb, Vs[li][sc * 128:(sc + 1) * 128, :])
            if li == 2:
                vr = pp.tile([128, D], BF16, tag=f"vr2_{sc}", bufs=1,
                             name=f"vr2_{sc}")
            else:
                vr = strm.tile([128, D], BF16, tag="vr", bufs=3, name="vr")
            if li == 0:
                nc.gpsimd.tensor_copy(vr, vsb)   # Pool is free early
            else:
                nc.scalar.copy(vr, vsb)          # ACT is free in the tail
            tiles.append(vr)
        vrs[li] = tiles

    def topk_read_phase(li, phase):
        scores_t, cands, scratches = phase
        S = SLOTS[li]
        nsc = S // 128
        probsT = []
        ws = []
        split = cands is not None
        top64s = []
        if scratches is None:
            scratches = [wp.tile([128, S], F32, tag=f"scratch{t}", bufs=2,
                                 name=f"scr_{rep}_{li}_{t}")
                         for t in range(TT)]
        for t in range(TT):
            top64s.append(sm.tile([128, 64], F32, tag=f"top64_{t}",
                                  name=f"top64_{t}"))

        if split:
            for t in range(TT):
                _rounds(nc, top64s[t], cands[t], cands[t])
        else:
            # quarter-chunks with top-32 each (P[Binom(64,1/4) > 32] ~ 1e-7
            # per row), then merge
            Q = S // 4
            for t in range(TT):
                cand = sm.tile([128, 128], F32, tag=f"cand{t}",
                               name=f"cand{t}")
                for qc in range(4):
                    seg = slice(qc * Q, (qc + 1) * Q)
                    _rounds(nc, cand[:, qc * 32:(qc + 1) * 32],
                            scores_t[t][:, seg], scratches[t][:, seg], nr=4)
                _rounds(nc, top64s[t], cand, cand)

        for t in range(TT):
            scores = scores_t[t]
            top64 = top64s[t]
            scratch = scratches[t]
            # t_sel = value at rank dyn_k (smallest selected score)
            selv = sm.tile([128, 64], F32, tag="selv", name="selv")
            nc.vector.select(selv, valid[t], top64, big)
            t_sel = sm.tile([128, 1], F32, tag="t_sel", name="t_sel")
            nc.vector.tensor_reduce(t_sel, selv, axis=AX.X, op=ALU.min)
            negm = sm.tile([128, 1], F32, tag="negm", name="negm")
            nc.vector.tensor_scalar_mul(negm, top64[:, 0:1], -1.0)

            # e^(s - max) straight into scratch (top-k rounds are done with
            # it), then mask in place: probs = (scores >= t_sel) * e.
            # Chunked so each probsT transpose group can start early.
            ng = max(1, S // 512)
            dparts = sm.tile([128, 4], F32, tag="dparts", name="dparts")
            for g in range(ng):
                seg = slice(g * 512, min((g + 1) * 512, S))
                nc.scalar.activation(scratch[:, seg], scores[:, seg], AF.Exp,
                                     bias=negm[:, 0:1])
                nc.vector.scalar_tensor_tensor(
                    out=scratch[:, seg], in0=scores[:, seg],
                    scalar=t_sel[:, 0:1], in1=scratch[:, seg],
                    op0=ALU.is_ge, op1=ALU.mult,
                    accum_out=dparts[:, g:g + 1])
            denom = sm.tile([128, 1], F32, tag="denom", name="denom")
            nc.vector.tensor_reduce(denom, dparts[:, :ng], axis=AX.X,
                                    op=ALU.add)
            rden = sm.tile([128, 1], F32, tag="rden", name="rden")
            nc.vector.reciprocal(rden, denom)
            w = sm.tile([128, 1], F32, tag=f"w{t}", name=f"w{t}")
            nc.vector.tensor_tensor(out=w, in0=rden,
                                    in1=route_sb[t][:, li:li + 1],
                                    op=ALU.mult)
            # fold route/denom into the probs so the read matmuls of all
            # three levels can accumulate in the same PSUM banks
            nc.vector.tensor_scalar(scratch, scratch, w[:, 0:1], None,
                                    op0=ALU.mult)

            # probsT (bf16, rounded on the psum->sbuf evacuation)
            pT = pp.tile([128, SLOTS[0] // 128, 128], BF16, tag=f"pt{t}",
                         name=f"pT{t}")
            for g in range(nsc // 4):
                pst = ps_tp.tile([128, 512], F32, tag="tp", name="pst")
                for j in range(4):
                    sc = g * 4 + j
                    nc.tensor.transpose(pst[:, j * 128:(j + 1) * 128],
                                        scratch[:, sc * 128:(sc + 1) * 128],
                                        ident)
                nc.vector.tensor_copy(pT[:, g * 4:(g + 1) * 4, :], pst)
            probsT.append(pT)

        # read matmul: probs (pre-scaled by route/denom) @ V, bf16;
        # all three levels accumulate in the same PSUM banks
        if li == 0:
            for t in range(TT):
                for dc2 in range(2):
                    prd[(t, dc2)] = ps_rd.tile(
                        [128, 512], F32, tag=f"rd{t}{dc2}",
                        name=f"prd{t}{dc2}")
        last = (li == len(SLOTS) - 1)
        for sc in range(nsc):
            for t in range(TT):
                for dc2 in range(2):
                    nc.tensor.matmul(prd[(t, dc2)], probsT[t][:, sc, :],
                                     vrs[li][sc][:, dc2 * 512:(dc2 + 1) * 512],
                                     start=(li == 0 and sc == 0),
                                     stop=(last and sc == nsc - 1))
        if last:
            for t in range(TT):
                for dc2 in range(2):
                    seg = slice(dc2 * 512, (dc2 + 1) * 512)
                    nc.scalar.copy(out_sb[t][:, seg], prd[(t, dc2)])

    # Pipeline: scores(L0) first so DVE's top-k starts ASAP; the router
    # (PE-heavy, DVE-free) then overlaps topk(L0); 1-level skew afterwards.
    vrs = {}
    prd = {}
    pending0 = scores_phase(0)
    router_phase()
    pending1 = scores_phase(1)
    pending2 = scores_phase(2)
    load_v(2)
    load_v(0)
    load_v(1)
    topk_read_phase(0, pending0)
    topk_read_phase(1, pending1)
    topk_read_phase(2, pending2)

    for t in range(TT):
        nc.sync.dma_start(out[t * 128:(t + 1) * 128, :], out_sb[t])


def build_nc(repeat=1):
    nc = bacc.Bacc(trn_type="TRN2", debug=False)

    env = {}
    env["q"] = nc.dram_tensor("q", [TOK_PER_CORE, D], F32,
                              kind="ExternalInput").ap()
    env["Ks"], env["Vs"], env["sals"] = [], [], []
    for i, S in enumerate(SLOTS):
        env["Ks"].append(
            nc.dram_tensor(f"K{i}", [S, D], F32, kind="ExternalInput").ap())
        env["Vs"].append(
            nc.dram_tensor(f"V{i}", [S, D], F32, kind="ExternalInput").ap())
        env["sals"].append(
            nc.dram_tensor(f"sal{i}", [S], F32, kind="ExternalInput").ap())
    env["rW1"] = nc.dram_tensor("rW1", [D, D], F32, kind="ExternalInput").ap()
    rb1 = nc.dram_tensor("rb1", [D], F32, kind="ExternalInput").ap()
    rW2 = nc.dram_tensor("rW2", [D, 3], F32, kind="ExternalInput").ap()
    rb2 = nc.dram_tensor("rb2", [3], F32, kind="ExternalInput").ap()
    pW1 = nc.dram_tensor("pW1", [D, 64], F32, kind="ExternalInput").ap()
    pb1 = nc.dram_tensor("pb1", [64], F32, kind="ExternalInput").ap()
    pW2 = nc.dram_tensor("pW2", [64, 1], F32, kind="ExternalInput").ap()
    pb2 = nc.dram_tensor("pb2", [1], F32, kind="ExternalInput").ap()
    env["out"] = nc.dram_tensor("out", [TOK_PER_CORE, D], F32,
                                kind="ExternalOutput").ap()

    with TileContext(nc) as tc:
        with (
            tc.tile_pool(name="const", bufs=1) as cp,
            tc.tile_pool(name="persist", bufs=1) as pp,
            tc.tile_pool(name="work", bufs=1) as wp,
            tc.tile_pool(name="stream", bufs=1) as strm,
            tc.tile_pool(name="small", bufs=1) as sm,
            tc.tile_pool(name="ps_tp", bufs=2, space="PSUM") as ps_tp,
            tc.tile_pool(name="ps_s", bufs=2, space="PSUM") as ps_s,
            tc.tile_pool(name="ps_rd", bufs=1, space="PSUM") as ps_rd,
        ):
            env.update(cp=cp, pp=pp, wp=wp, strm=strm, sm=sm, ps_tp=ps_tp,
                       ps_s=ps_s, ps_rd=ps_rd, ps_sm=ps_s)
            # ---------------- constants ----------------
            ident = cp.tile([128, 128], F32)
            make_identity(nc, ident)
            ones = cp.tile([65, 128], F32)
            nc.vector.memset(ones, 1.0)
            env["ones_at"] = lambda p: ones[p:p + 1, :]
            iota_i = cp.tile([128, 64], mybir.dt.int32)
            nc.gpsimd.iota(iota_i, pattern=[[1, 64]], base=1,
                           channel_multiplier=0)
            iota_f = cp.tile([128, 64], F32)
            nc.vector.tensor_copy(iota_f, iota_i)
            nc.vector.memset(iota_f[:, 0:1], NEG_BIG)
            big = cp.tile([128, 64], F32)
            nc.vector.memset(big, 1.0e30)

            # rb1 packed into salpack row 32, cols 1024:2048 (sal1 uses 0:1024)
            rb2_row = cp.tile([1, 3], F32)
            nc.sync.dma_start(rb2_row, rb2.unsqueeze(0))
            pb1_row = cp.tile([1, 64], F32)
            nc.sync.dma_start(pb1_row, pb1.unsqueeze(0))
            pb2_row = cp.tile([1, 1], F32)
            nc.sync.dma_start(pb2_row, pb2.unsqueeze(0))

            # rows 0/32/64: matmul rhs base_partition must be 0/32/64
            salpack = cp.tile([65, SLOTS[0]], F32, name="salpack")
            for _i, _S in enumerate(SLOTS):
                nc.sync.dma_start(salpack[_i * 32:_i * 32 + 1, :_S],
                                  env["sals"][_i].unsqueeze(0))
            env["sal_rows"] = [salpack[_i * 32:_i * 32 + 1, :]
                               for _i in range(3)]
            nc.sync.dma_start(salpack[32:33, 1024:1024 + D], rb1.unsqueeze(0))
            env["rb1_row"] = salpack[32:33, 1024:1024 + D]

            pW1_sb = cp.tile([128, 8, 64], F32)
            nc.sync.dma_start(pW1_sb, pW1.rearrange("(c p) o -> p c o", p=128))
            pW2_sb = cp.tile([64, 1], F32)
            nc.sync.dma_start(pW2_sb, pW2)
            rW2_sb = cp.tile([128, 8, 3], F32)
            nc.sync.dma_start(rW2_sb, rW2.rearrange("(c p) o -> p c o", p=128))

            env.update(ident=ident, ones=ones, iota_f=iota_f, big=big,
                       rb2_row=rb2_row, pb1_row=pb1_row,
                       pb2_row=pb2_row, pW1_sb=pW1_sb, pW2_sb=pW2_sb,
                       rW2_sb=rW2_sb)

            # persistent per-token-tile state
            env["qT"] = [pp.tile([128, 8, 128], F32, tag=f"qT{t}",
                                 name=f"qT{t}") for t in range(TT)]
            env["out_sb"] = [pp.tile([128, D], F32, tag=f"out{t}",
                                     name=f"out_sb{t}") for t in range(TT)]
            env["route_sb"] = [pp.tile([128, 3], F32, tag=f"route{t}",
                                       name=f"route{t}") for t in range(TT)]
            env["valid"] = [pp.tile([128, 64], U8, tag=f"valid{t}",
                                    name=f"valid{t}") for t in range(TT)]

            for rep in range(repeat):
                _emit_body(nc, env, rep)

    nc.compile()
    return nc


_NC_CACHE = None


def _get_nc():
    global _NC_CACHE
    if _NC_CACHE is None:
        _NC_CACHE = build_nc()
    return _NC_CACHE


def make_in_maps(inputs):
    q_full = np.ascontiguousarray(
        np.asarray(inputs["q"], dtype=np.float32).reshape(B * T, D))
    shared = {}
    for name in ["K0", "V0", "sal0", "K1", "V1", "sal1", "K2", "V2", "sal2",
                 "rW1", "rb1", "rW2", "rb2", "pW1", "pb1", "pW2", "pb2"]:
        shared[name] = np.ascontiguousarray(
            np.asarray(inputs[name], dtype=np.float32))
    in_maps = []
    for c in range(N_CORES):
        m = dict(shared)
        m["q"] = np.ascontiguousarray(
            q_full[c * TOK_PER_CORE:(c + 1) * TOK_PER_CORE])
        in_maps.append(m)
    return in_maps


def kernel(**inputs):
    nc = _get_nc()
    in_maps = make_in_maps(inputs)
    res = run_bass_kernel_spmd(nc, in_maps, core_ids=list(range(N_CORES)))
    out = np.concatenate([res.results[c]["out"] for c in range(N_CORES)],
                         axis=0)
    return out.reshape(B, T, D)

